# revision 1
# baseline (speedup 1.0000x reference)
"""AttnBlock++1d Trainium2 kernel.

B=8, C=512, T=1024, H=8 heads (Ch=64), 32 groupnorm groups.
Sharding: data-parallel over batch, one batch element per NeuronCore (8 cores).

Per-core design (all matmuls bf16 operands / fp32 PSUM accumulation):
  - GroupNorm: per-channel (mean, var) via bn_stats/bn_aggr on DVE;
    a block-diagonal [128,128] averaging matmul
    aggregates over each 16-channel group AND broadcasts back to channels in
    one shot; rsqrt = ACT sqrt + exact reciprocal + one Newton step; applied
    with a single tensor_scalar (scale/bias per partition) emitting bf16 h.
  - q = (0.125*W0)^T h + 0.125*b0, k = W1^T h + b1 in [C, T] layout;
    v^T = h^T W2 in [T, C] layout (so attention needs no transposes).
  - Scores computed transposed, S^T[i,t] = k^T q per head, two heads row-
    packed per matmul (K=64 at base partitions 0/64); softmax without
    max-subtraction (scores are O(30), exp stays in fp32 range); exp on ACT
    from two alternating 2-bank PSUM tiles so ACT never waits on PE; E bf16.
  - AV: a[c,t] = sum_i vT_aug[i, c|1]^T E[i,t] with a ones column appended
    to v^T so row 64 of the PSUM output is the softmax denominator. AV
    matmuls trail the score/exp stream (software pipeline across pairs);
    the vT and q/k m=1..3 projections are emitted as PE fillers under
    pair 0 while the AV PSUM pool is still closed.
  - Normalize: reciprocal_approx_fast + DRAM-bounce partition broadcast of
    the sums row, tensor_tensor multiply, + b2 (softmax weights sum to 1 so
    the v bias is a plain post-add). Non-final pairs stage AV out of PSUM
    first so the banks recycle; the last pair pipelines per-head halves.
  - out = (x + b3) + W3^T a, with x+b3 folded on host; NIN3 accumulators
    reuse the freed S^T/AV PSUM slots, k-tiles 0..2 first so they overlap
    the last pair's normalize chain.
"""

import numpy as np
import ml_dtypes

B, C, T = 8, 512, 1024
H = 8
CH = C // H  # 64
G = 32  # groupnorm groups
GS = C // G  # 16 channels per group
EPS = 1e-6
NT = C // 128  # 4 channel tiles
IT = T // 128  # 8 i-tiles
NCORES = 8

_bf16 = ml_dtypes.bfloat16


def _build_nc():
    import concourse.bass as bass
    import concourse.tile as tile
    from concourse import bacc, mybir

    f32 = mybir.dt.float32
    bf16 = mybir.dt.bfloat16
    AF = mybir.ActivationFunctionType
    OP = mybir.AluOpType

    nc = bacc.Bacc("TRN2", target_bir_lowering=False, debug=False)

    x_d = nc.dram_tensor("x", [C, T], f32, kind="ExternalInput").ap()
    xb3_d = nc.dram_tensor("xb3", [C, T], f32, kind="ExternalInput").ap()
    w_d = [
        nc.dram_tensor(f"w{i}", [C, C], bf16, kind="ExternalInput").ap()
        for i in range(4)
    ]
    bqk_d = nc.dram_tensor("bqk", [128, 2 * NT], f32, kind="ExternalInput").ap()
    b2h_d = nc.dram_tensor("b2h", [CH, H], f32, kind="ExternalInput").ap()
    gb_d = nc.dram_tensor("gb", [128, 2 * NT], f32, kind="ExternalInput").ap()
    p_d = nc.dram_tensor("pmat", [128, 128], f32, kind="ExternalInput").ap()
    out_d = nc.dram_tensor("out", [C, T], f32, kind="ExternalOutput").ap()

    with tile.TileContext(nc) as tc:
        _emit(nc, tc, bass, mybir, f32, bf16, AF, OP,
              x_d, xb3_d, w_d, bqk_d, b2h_d, gb_d, p_d, out_d)
    nc.compile()
    return nc


def _emit(nc, tc, bass, mybir, f32, bf16, AF, OP,
          x_d, xb3_d, w_d, bqk_d, b2h_d, gb_d, p_d, out_d):
    from contextlib import ExitStack

    ctx = ExitStack()
    with ctx:
        persist = ctx.enter_context(tc.tile_pool(name="persist", bufs=1))
        small = ctx.enter_context(tc.tile_pool(name="small", bufs=2))
        dram = ctx.enter_context(tc.tile_pool(name="dram", bufs=2, space="DRAM"))

        # ---- persistent SBUF tiles ----
        # x is only needed for GroupNorm; scope it so its SBUF is reused by
        # the attention-phase pools (xpool closes after the GN apply).
        xpool_ctx = ExitStack()
        xpool = xpool_ctx.enter_context(tc.tile_pool(name="xpool", bufs=1))
        x_sb = xpool.tile([128, NT * T], f32, tag="x")
        xb3_sb = persist.tile([128, NT * T], f32, tag="xb3")
        h_sb = persist.tile([128, NT * T], bf16, tag="h")
        q_sb = persist.tile([128, NT * T], bf16, tag="q")
        k_sb = persist.tile([128, NT * T], bf16, tag="k")
        vt_sb = persist.tile([128, IT * H * (CH + 1)], bf16, tag="vt")  # 8*520
        # one `a` tile per head-pair so NIN3's early k-tiles don't pick up
        # false whole-tile dependencies on the last pair's normalize writes
        a_sb = [persist.tile([128, T], bf16, tag=f"a{p}", name=f"a{p}")
                for p in range(NT)]
        w_sb = [[persist.tile([128, C], bf16, tag=f"w{i}_{j}", name=f"w{i}_{j}")
                 for j in range(NT)] for i in range(4)]
        p_sb = persist.tile([128, 128], f32, tag="pmat")
        bqk_sb = persist.tile([128, 2 * NT], f32, tag="bqk")
        b2h_sb = persist.tile([CH, H], f32, tag="b2h")
        gb_sb = persist.tile([128, 2 * NT], f32, tag="gb")
        scb_sb = persist.tile([128, 2 * NT], f32, tag="scb")

        # ---- input DMAs ----
        for j in range(NT):
            nc.sync.dma_start(x_sb[:, j * T:(j + 1) * T], x_d[j * 128:(j + 1) * 128, :])
        nc.sync.dma_start(p_sb[:], p_d[:])
        nc.sync.dma_start(bqk_sb[:], bqk_d[:])
        nc.sync.dma_start(b2h_sb[:], b2h_d[:])
        nc.sync.dma_start(gb_sb[:], gb_d[:])
        for i in range(3):
            for j in range(NT):
                nc.sync.dma_start(w_sb[i][j][:], w_d[i][j * 128:(j + 1) * 128, :])

        # =================== GroupNorm ===================
        gn_ctx = ExitStack()
        gn_ps = gn_ctx.enter_context(tc.tile_pool(name="gn_ps", bufs=1, space="PSUM"))

        # Per-channel sum(x) on DVE and sum(x^2) on ACT (Square + accum_out)
        # so the two run in parallel per tile as the x DMAs land. The group
        # matmul folds the 1/(16*1024) mean divisor, giving per-channel
        # group (mean, E[x^2]) directly.
        me_cols = persist.tile([128, 2 * NT], f32, tag="me")
        for j in range(NT):
            sqscr = small.tile([128, T], bf16, tag="sqscr")
            nc.scalar.activation(sqscr[:], x_sb[:, j * T:(j + 1) * T], AF.Square,
                                 accum_out=me_cols[:, 2 * j + 1:2 * j + 2])
            nc.vector.reduce_sum(out=me_cols[:, 2 * j:2 * j + 1],
                                 in_=x_sb[:, j * T:(j + 1) * T],
                                 axis=mybir.AxisListType.X)

        # Group-aggregate AND broadcast back to channels in one matmul per
        # tile: P[c, c'] = 1/(16*1024) if same group. Output = per-channel
        # group (mean, E[x^2]), already replicated across each group.
        ge_sb = persist.tile([128, 2 * NT], f32, tag="ge")
        for j in range(NT):
            ge_ps = gn_ps.tile([128, 2], f32, tag="ge_ps")
            nc.tensor.matmul(ge_ps[:], p_sb[:], me_cols[:, 2 * j:2 * j + 2],
                             start=True, stop=True)
            nc.vector.tensor_copy(ge_sb[:, 2 * j:2 * j + 2], ge_ps[:])

        # batched over the 4 tiles with stride-2 column views [128, NT]
        ge_v = ge_sb[:].rearrange("p (j s) -> p s j", s=2)
        mu_all, e_all = ge_v[:, 0, :], ge_v[:, 1, :]
        veps = small.tile([128, NT], f32, tag="veps")
        nc.vector.tensor_tensor(out=veps[:], in0=mu_all, in1=mu_all, op=OP.mult)
        nc.vector.tensor_tensor(out=veps[:], in0=e_all, in1=veps[:], op=OP.subtract)
        nc.vector.tensor_scalar_add(out=veps[:], in0=veps[:], scalar1=float(EPS))
        # rsig = rsqrt(veps) with one Newton refinement
        sig = small.tile([128, NT], f32, tag="sig")
        nc.scalar.activation(sig[:], veps[:], AF.Sqrt)
        rsig0 = small.tile([128, NT], f32, tag="rsig0")
        nc.vector.reciprocal(rsig0[:], sig[:])
        tnw = small.tile([128, NT], f32, tag="tnw")
        nc.vector.tensor_tensor(out=tnw[:], in0=rsig0[:], in1=rsig0[:], op=OP.mult)
        nc.vector.tensor_tensor(out=tnw[:], in0=tnw[:], in1=veps[:], op=OP.mult)
        nc.vector.tensor_scalar(out=tnw[:], in0=tnw[:], scalar1=-0.5, scalar2=1.5,
                                op0=OP.mult, op1=OP.add)
        rsig = small.tile([128, NT], f32, tag="rsig")
        nc.vector.tensor_tensor(out=rsig[:], in0=rsig0[:], in1=tnw[:], op=OP.mult)

        # preload the exp ACT table set during QKV so pair 0 doesn't stall
        dume = small.tile([1, 1], f32, tag="dume")
        nc.scalar.activation(dume[:], sig[0:1, 0:1], AF.Exp)

        # scale = gamma * rsig ; bias = beta - mu * scale  (strided scb writes)
        scb_v = scb_sb[:].rearrange("p (j s) -> p s j", s=2)
        scale_cols, bias_cols = scb_v[:, 0, :], scb_v[:, 1, :]
        nc.vector.tensor_tensor(out=scale_cols, in0=gb_sb[:, 0:NT], in1=rsig[:],
                                op=OP.mult)
        tmu = small.tile([128, NT], f32, tag="tmu")
        nc.vector.tensor_tensor(out=tmu[:], in0=mu_all, in1=scale_cols, op=OP.mult)
        nc.vector.tensor_tensor(out=bias_cols, in0=gb_sb[:, NT:2 * NT], in1=tmu[:],
                                op=OP.subtract)

        # apply: h = x * scale + bias (bf16 out)
        for j in range(NT):
            nc.vector.tensor_scalar(
                out=h_sb[:, j * T:(j + 1) * T], in0=x_sb[:, j * T:(j + 1) * T],
                scalar1=scb_sb[:, 2 * j:2 * j + 1], scalar2=scb_sb[:, 2 * j + 1:2 * j + 2],
                op0=OP.mult, op1=OP.add)

        gn_ctx.close()

        xpool_ctx.close()
        # attention/output SBUF pools open now, reusing x's space
        epool = ctx.enter_context(tc.tile_pool(name="epool", bufs=10))
        atsm = ctx.enter_context(tc.tile_pool(name="atsm", bufs=2))
        ostp = ctx.enter_context(tc.tile_pool(name="ostp", bufs=2))

        # =================== QKV projections ===================
        # at_ps (4 banks: two 2-bank S^T slots) opens BEFORE the QKV pools so
        # it gets disjoint PSUM banks -- attention scores can then start while
        # QKV projections are still draining. QKV (incl. vT, folded into the
        # same tag) fits in the other 4 banks.
        at_ctx = ExitStack()
        at_ps = at_ctx.enter_context(tc.tile_pool(name="at_ps", bufs=1, space="PSUM"))
        qkv_ctx = ExitStack()
        qkv_ps = qkv_ctx.enter_context(tc.tile_pool(name="qkv_ps", bufs=2, space="PSUM"))
        vp_ps = qkv_ps

        def _proj_half(wi, dst, bcol0, m, ch, state):
            if ch == 0:
                state["pp"] = qkv_ps.tile([128, T], f32, tag="qk", name="pp")
            pp = state["pp"]
            for kk in range(NT):
                nc.tensor.matmul(
                    pp[:, ch * 512:(ch + 1) * 512],
                    w_sb[wi][kk][:, m * 128:(m + 1) * 128],
                    h_sb[:, kk * T + ch * 512: kk * T + (ch + 1) * 512],
                    start=(kk == 0), stop=(kk == NT - 1))
            if ch == 1:
                nc.vector.tensor_scalar_add(
                    out=dst[:, m * T:(m + 1) * T], in0=pp[:],
                    scalar1=bqk_sb[:, bcol0 + m: bcol0 + m + 1])

        def _proj(wi, dst, bcol0, m):
            state = {}
            _proj_half(wi, dst, bcol0, m, 0, state)
            _proj_half(wi, dst, bcol0, m, 1, state)

        # m=0 of q/k first so pair-0 attention can start early.
        _proj(0, q_sb, 0, 0)
        _proj(1, k_sb, NT, 0)

        # w3 and xb3 are only needed by NIN3 at the end; load them after the
        # startup-critical x/w0/w1/w2 traffic.
        for j in range(NT):
            nc.sync.dma_start(w_sb[3][j][:], w_d[3][j * 128:(j + 1) * 128, :])
        for j in range(NT):
            nc.sync.dma_start(xb3_sb[:, j * T:(j + 1) * T],
                              xb3_d[j * 128:(j + 1) * 128, :])

        # =================== Attention (per head pair) ===================
        AUG = CH + 1  # 65

        av_tiles = {}

        def _bcast_recip(r_src, width, name_sfx):
            r_sb = atsm.tile([1, width], f32, tag=f"r_sb{name_sfx}", bufs=1,
                             name="r_sb")
            nc.vector.reciprocal_approx_fast(out=r_sb[:], in_=r_src)
            scr = dram.tile([1, width], f32, tag=f"rscr{name_sfx}", name="scr")
            nc.sync.dma_start(scr[:], r_sb[:])
            r_bc = atsm.tile([CH, width], f32, tag=f"r_bc{name_sfx}", bufs=1, name="r_bc")
            nc.gpsimd.dma_start(
                r_bc[:], bass.AP(tensor=scr.tensor, offset=scr[:].offset,
                                 ap=[[0, CH], [1, width]]))
            return r_bc

        def _norm_tail(p, av):
            h0, h1 = 2 * p, 2 * p + 1
            if p < H // 2 - 1:
                # Stage the AV result and its sums row out of PSUM so the av
                # banks free up for the next pair before the reciprocal's
                # DRAM broadcast round-trip.
                sums_sb = atsm.tile([1, 2 * T], f32, tag="sums", bufs=1,
                                    name="sums_sb")
                nc.vector.tensor_copy(sums_sb[:], av[CH:CH + 1, :])
                stage = atsm.tile([CH, 2 * T], f32, tag="stage", bufs=1, name="stage")
                nc.vector.tensor_copy(stage[:], av[0:CH, :])
                r_bc = _bcast_recip(sums_sb[:], 2 * T, "")
                an = atsm.tile([CH, 2 * T], bf16, tag="an", bufs=1, name="an")
                nc.vector.tensor_tensor(out=an[:], in0=stage[:], in1=r_bc[:],
                                        op=OP.mult)
                # h1 (partition-shift DMA, the longer path) first, then h0
                # which writes its a_sb rows directly.
                stage2 = atsm.tile([CH, T], bf16, tag="stage2", name="stage2")
                nc.vector.tensor_scalar_add(out=stage2[:], in0=an[:, T:2 * T],
                                            scalar1=b2h_sb[:, h1:h1 + 1])
                nc.sync.dma_start(a_sb[p][CH:2 * CH, :], stage2[:])
                nc.vector.tensor_scalar_add(out=a_sb[p][0:CH, :],
                                            in0=an[:, 0:T],
                                            scalar1=b2h_sb[:, h0:h0 + 1])
                return

            # Last pair: this chain is fully exposed on the critical path, so
            # normalize straight out of PSUM and pipeline the two head-halves
            # (h1's partition-shift DMA first) through recip/broadcast/mult.
            # The sums row must bounce through SBUF: reciprocal_approx_fast
            # (custom DVE uop) misreads PSUM operands on hardware.
            sumsL = atsm.tile([1, 2 * T], f32, tag="sumsL", bufs=1, name="sumsL")
            nc.vector.tensor_copy(sumsL[:], av[CH:CH + 1, :])
            rA = _bcast_recip(sumsL[:, T:2 * T], T, "A")
            rB = _bcast_recip(sumsL[:, 0:T], T, "B")
            anA = atsm.tile([CH, T], bf16, tag="anA", bufs=1, name="anA")
            nc.vector.tensor_tensor(out=anA[:], in0=av[0:CH, T:2 * T], in1=rA[:],
                                    op=OP.mult)
            stage2 = atsm.tile([CH, T], bf16, tag="stage2", name="stage2")
            nc.vector.tensor_scalar_add(out=stage2[:], in0=anA[:],
                                        scalar1=b2h_sb[:, h1:h1 + 1])
            nc.sync.dma_start(a_sb[p][CH:2 * CH, :], stage2[:])
            anB = atsm.tile([CH, T], bf16, tag="anB", bufs=1, name="anB")
            nc.vector.tensor_tensor(out=anB[:], in0=av[0:CH, 0:T], in1=rB[:],
                                    op=OP.mult)
            nc.vector.tensor_scalar_add(out=a_sb[p][0:CH, :], in0=anB[:],
                                        scalar1=b2h_sb[:, h0:h0 + 1])

        def _av_mms(p, itx, eA, eB):
            if itx == 0:
                av_tiles[p] = av_ps.tile([AUG, 2 * T], f32, tag="av", name="av")
            av = av_tiles[p]
            base = itx * H * AUG
            lh0 = vt_sb[:, base + 2 * p * AUG: base + 2 * p * AUG + AUG]
            lh1 = vt_sb[:, base + (2 * p + 1) * AUG: base + (2 * p + 1) * AUG + AUG]
            st_, sp_ = (itx == 0), (itx == IT - 1)
            nc.tensor.matmul(av[:, 0:512], lh0, eA[:, 0:512], start=st_, stop=sp_)
            nc.tensor.matmul(av[:, 512:1024], lh0, eB[:, 0:512], start=st_, stop=sp_)
            nc.tensor.matmul(av[:, 1024:1536], lh1, eA[:, 512:1024],
                             start=st_, stop=sp_)
            nc.tensor.matmul(av[:, 1536:2048], lh1, eB[:, 512:1024],
                             start=st_, stop=sp_)

        def _normalize(p):
            _norm_tail(p, av_tiles.pop(p))

        def _st_exp(p, it):
            # Two alternating 2-bank S^T tiles (chunk 0 / chunk 1) so the ACT
            # exp of one overlaps the PE scores-matmuls of the other.
            kslc = slice(p * T + it * 128, p * T + (it + 1) * 128)
            eAB = []
            for ch in range(2):
                stc = at_ps.tile([128, T], f32, tag=f"st{ch}", name=f"st{ch}")
                qslc = slice(p * T + ch * 512, p * T + (ch + 1) * 512)
                nc.tensor.matmul(stc[:, 0:512], k_sb[0:64, kslc],
                                 q_sb[0:64, qslc], start=True, stop=True)
                nc.tensor.matmul(stc[:, 512:1024], k_sb[64:128, kslc],
                                 q_sb[64:128, qslc], start=True, stop=True)
                ec = epool.tile([128, T], bf16, tag="E", name="ec", bufs=24)
                nc.scalar.activation(ec[:], stc[:], AF.Exp)
                eAB.append(ec)
            return eAB

        def _flush(pend):
            pp_, pit, eA, eB = pend.popleft()
            _av_mms(pp_, pit, eA, eB)
            if pit == IT - 1:
                _normalize(pp_)

        # Software pipeline across the whole attention phase. During pair 0
        # (av pool not yet open, so the QKV PSUM banks are still available)
        # the vT projection and the q/k m=1..3 projections are emitted as PE
        # fillers underneath the score/exp stream. AV matmuls for pair 0
        # flush right after; later pairs trail by ~2 steps.
        from collections import deque
        vt_v = vt_sb[:].rearrange("p (i h c) -> p i h c", i=IT, h=H)
        nc.vector.memset(vt_sb[:], 1.0)

        def _vp_unit(it):
            vp = vp_ps.tile([128, C], f32, tag="qk", name="vp")
            for kk in range(NT):
                nc.tensor.matmul(
                    vp[:], h_sb[:, kk * T + it * 128: kk * T + (it + 1) * 128],
                    w_sb[2][kk][:], start=(kk == 0), stop=(kk == NT - 1))
            nc.vector.tensor_copy(vt_v[:, it, :, 0:CH],
                                  vp[:].rearrange("p (h c) -> p h c", h=H))

        fillers = deque()
        for it in range(IT):
            fillers.append(lambda it=it: _vp_unit(it))
        for m in range(1, NT):
            for wi, dst, bcol0 in ((0, q_sb, 0), (1, k_sb, NT)):
                state = {}
                for ch in range(2):
                    fillers.append(
                        lambda wi=wi, dst=dst, b=bcol0, m=m, ch=ch, s=state:
                        _proj_half(wi, dst, b, m, ch, s))

        pend = deque()
        for it in range(IT):
            pend.append((0, it, *_st_exp(0, it)))
            for _ in range(2):
                if fillers:
                    fillers.popleft()()
        while fillers:
            fillers.popleft()()
        qkv_ctx.close()
        av_ps = at_ctx.enter_context(
            tc.tile_pool(name="av_ps", bufs=1, space="PSUM"))

        for p in range(1, H // 2):
            for it in range(IT):
                pend.append((p, it, *_st_exp(p, it)))
                _flush(pend)
                if len(pend) > 3:
                    _flush(pend)
        while pend:
            _flush(pend)

        # =================== NIN3 + residual ===================
        # Reuse the freed S^T PSUM slots (2 banks each) for the NIN3
        # accumulators. kk=0..2 accumulate while the last pair's normalize
        # chain is still in flight (keeps the PE p-state warm); kk=3 lands
        # once the last pair's `a` tile is written.
        def _nin_mms(pp, m, kks, start, stop):
            for ch in range(2):
                for kk in kks:
                    nc.tensor.matmul(
                        pp[:, ch * 512:(ch + 1) * 512],
                        w_sb[3][kk][:, m * 128:(m + 1) * 128],
                        a_sb[kk][:, ch * 512:(ch + 1) * 512],
                        start=(start and kk == kks[0]),
                        stop=(stop and kk == kks[-1]))

        def _nin_tail(pp, m):
            _nin_mms(pp, m, [3], start=False, stop=True)
            ost = ostp.tile([128, T], f32, tag="ost", name="ost")
            nc.vector.tensor_tensor(out=ost[:], in0=pp[:],
                                    in1=xb3_sb[:, m * T:(m + 1) * T], op=OP.add)
            eng = nc.sync if m % 2 == 0 else nc.gpsimd
            eng.dma_start(out_d[m * 128:(m + 1) * 128, :], ost[:])

        nin = {}
        nin_pools = {0: (at_ps, "st0"), 1: (at_ps, "st1"),
                     2: (av_ps, "av"), 3: (at_ps, "st0")}
        for m in (0, 1, 2):
            pool, tg = nin_pools[m]
            nin[m] = pool.tile([128, T], f32, tag=tg, name=f"nin{m}")
            _nin_mms(nin[m], m, [0, 1, 2], start=True, stop=False)
        for m in (0, 1, 2):
            _nin_tail(nin[m], m)
        pool, tg = nin_pools[3]
        nin[3] = pool.tile([128, T], f32, tag=tg, name="nin3")
        _nin_mms(nin[3], 3, [0, 1, 2], start=True, stop=False)
        _nin_tail(nin[3], 3)
        at_ctx.close()


def _host_inputs(inputs):
    """Build the per-core in_maps from the full problem inputs."""
    x = np.ascontiguousarray(inputs["x"], dtype=np.float32)
    gamma = np.asarray(inputs["gamma"], dtype=np.float32)
    beta = np.asarray(inputs["beta"], dtype=np.float32)
    scale = np.float32(CH ** -0.5)  # 0.125, exact power of two

    w0 = (np.asarray(inputs["W0"], dtype=np.float32) * scale).astype(_bf16)
    w1 = np.asarray(inputs["W1"], dtype=np.float32).astype(_bf16)
    w2 = np.asarray(inputs["W2"], dtype=np.float32).astype(_bf16)
    w3 = np.asarray(inputs["W3"], dtype=np.float32).astype(_bf16)

    b0 = np.asarray(inputs["b0"], dtype=np.float32) * scale
    b1 = np.asarray(inputs["b1"], dtype=np.float32)
    b2 = np.asarray(inputs["b2"], dtype=np.float32)
    b3 = np.asarray(inputs["b3"], dtype=np.float32)

    bqk = np.concatenate([b0.reshape(NT, 128).T, b1.reshape(NT, 128).T], axis=1)
    bqk = np.ascontiguousarray(bqk, dtype=np.float32)  # [128, 8]
    b2h = np.ascontiguousarray(b2.reshape(H, CH).T, dtype=np.float32)  # [64, 8]

    # block-diagonal group-averaging matrix: P[c, c'] = 1/16 if same group
    cc = np.arange(128)
    pmat = (cc[:, None] // GS == cc[None, :] // GS).astype(np.float32) / (GS * T)

    gb = np.concatenate([gamma.reshape(NT, 128).T, beta.reshape(NT, 128).T], axis=1)
    gb = np.ascontiguousarray(gb, dtype=np.float32)  # [128, 8]

    common = {
        "w0": w0, "w1": w1, "w2": w2, "w3": w3,
        "bqk": bqk, "b2h": b2h, "gb": gb, "pmat": pmat,
    }
    in_maps = []
    for b in range(NCORES):
        m = dict(common)
        m["x"] = np.ascontiguousarray(x[b])
        m["xb3"] = np.ascontiguousarray(x[b] + b3[:, None])
        in_maps.append(m)
    return in_maps


def kernel(**inputs) -> np.ndarray:
    from concourse.bass_utils import run_bass_kernel_spmd

    nc = _build_nc()
    in_maps = _host_inputs(inputs)
    res = run_bass_kernel_spmd(nc, in_maps, core_ids=list(range(NCORES)))
    out = np.stack([np.asarray(r["out"], dtype=np.float32) for r in res.results])
    return out

